# revision 1
# baseline (speedup 1.0000x reference)
"""Trainium2 Bass kernel for nn_BiasWeightLayerPrime.

Computes out[b, n] = x[b, n] * w[n] + v[n] where
    w[n] = sum_p kernel[p, n mod prime_p],  v[n] = sum_p bias[p, n mod prime_p]
over the 168 primes below 1000.

Distribution: the flattened feature axis N = 524288 is sharded across the
8 NeuronCores (65536 columns each); the batch (64) is kept whole per core.

Per core the shard is processed in 16 DMA tiles of (128, 2048) fp32 (1 MiB):
partitions 0..63 hold the 64 batch rows for one 2048-column block, partitions
64..127 the next block. The host pre-permutes x into this tile-major layout
(and inverse-permutes the output) so every DMA reads/writes contiguous DRAM —
measured 360 GB/s vs 140 GB/s for the strided row-major access pattern.

Per 1024-column compute sub-tile, the tiny per-tile w/bias slices are
broadcast across the two 64-partition halves by the PE with a constant
(6, 128) 0/1 selector matmul into PSUM. The matmuls run in bf16 at full PE
rate using an exact 3-limb decomposition (hi+mid+lo bf16 == fp32 bitwise
after the fp32 PSUM accumulate). DVE then computes y = x * w_bcast and
y += b_bcast (two fp32 tensor_tensor ops), and the tile is stored back.
All large transfers use nc.gpsimd (SWDGE, sprays all 16 SDMA engines);
the HWDGE ring only drives 2 engines and tops out near 52 GB/s.
"""

import os

import numpy as np

from concourse import bacc, mybir
import concourse.bass as bass
import concourse.tile as tile
from concourse.bass_utils import run_bass_kernel_spmd

N_CORES = 8
B = 64
N_FULL = 524288
S = N_FULL // N_CORES   # 65536 columns per core
F = 1024                # compute sub-tile width
W = 2048                # per-partition elements per DMA tile (1 MiB tiles)
NBIG = S // (2 * W)     # DMA tiles per core (16)
NSUB = W // F           # compute sub-tiles per DMA tile (2)
NTILES = S // (2 * F)   # compute sub-tiles per core (32)

_PRIMES = [
    2, 3, 5, 7, 11, 13, 17, 19, 23, 29, 31, 37, 41, 43, 47, 53, 59, 61, 67,
    71, 73, 79, 83, 89, 97, 101, 103, 107, 109, 113, 127, 131, 137, 139, 149,
    151, 157, 163, 167, 173, 179, 181, 191, 193, 197, 199, 211, 223, 227, 229,
    233, 239, 241, 251, 257, 263, 269, 271, 277, 281, 283, 293, 307, 311, 313,
    317, 331, 337, 347, 349, 353, 359, 367, 373, 379, 383, 389, 397, 401, 409,
    419, 421, 431, 433, 439, 443, 449, 457, 461, 463, 467, 479, 487, 491, 499,
    503, 509, 521, 523, 541, 547, 557, 563, 569, 571, 577, 587, 593, 599, 601,
    607, 613, 617, 619, 631, 641, 643, 647, 653, 659, 661, 673, 677, 683, 691,
    701, 709, 719, 727, 733, 739, 743, 751, 757, 761, 769, 773, 787, 797, 809,
    811, 821, 823, 827, 829, 839, 853, 857, 859, 863, 877, 881, 883, 887, 907,
    911, 919, 929, 937, 941, 947, 953, 967, 971, 977, 983, 991, 997,
]


def _prime_mask(table: np.ndarray, n: int) -> np.ndarray:
    """w[j] = sum_p table[p, j mod prime_p] for j in [0, n) — float64 accum."""
    acc = np.zeros(n, dtype=np.float64)
    for i, p in enumerate(_PRIMES):
        row = table[i, :p].astype(np.float64)
        reps = -(-n // p)
        acc += np.tile(row, reps)[:n]
    return acc.astype(np.float32)


def build_bass(s=S, f=F, w_run=W):
    """Build the single-core Bass program for a shard of s columns."""
    nbig = s // (2 * w_run)
    nsub = w_run // f
    ntiles = nbig * nsub
    PREFETCH = 4

    nc = bacc.Bacc("TRN2", target_bir_lowering=False, debug=False)
    dt = mybir.dt.float32
    bf = mybir.dt.bfloat16
    x = nc.dram_tensor("x", (nbig, 128, w_run), dt, kind="ExternalInput")
    wb = nc.dram_tensor("wb", (ntiles, 6, 2 * f), bf, kind="ExternalInput")
    sel = nc.dram_tensor("sel", (6, 128), bf, kind="ExternalInput")
    out = nc.dram_tensor("out", (nbig, 128, w_run), dt, kind="ExternalOutput")

    with tile.TileContext(nc) as tc:
        with (
            tc.tile_pool(name="xp", bufs=PREFETCH + 2) as xp,
            tc.tile_pool(name="yp", bufs=4) as yp,
            tc.tile_pool(name="wbp", bufs=6) as wbp,
            tc.tile_pool(name="selp", bufs=1) as selp,
            tc.tile_pool(name="psw", bufs=2, space=bass.MemorySpace.PSUM) as psw,
            tc.tile_pool(name="psb", bufs=2, space=bass.MemorySpace.PSUM) as psb,
        ):
            sel_t = selp.tile([6, 128], bf)
            nc.sync.dma_start(sel_t[:], sel.ap())

            def load_x(bt):
                xt = xp.tile([128, w_run], dt)
                nc.gpsimd.dma_start(xt[:], x.ap()[bt])
                return xt

            xts = {bt: load_x(bt) for bt in range(min(PREFETCH, nbig))}

            for bt in range(nbig):
                if bt + PREFETCH < nbig:
                    xts[bt + PREFETCH] = load_x(bt + PREFETCH)
                xt = xts.pop(bt)

                yt = yp.tile([128, w_run], dt)
                for s2 in range(nsub):
                    t = bt * nsub + s2
                    wbt = wbp.tile([6, 2 * f], bf)
                    nc.sync.dma_start(wbt[:], wb.ap()[t])

                    pw = psw.tile([128, f], dt)
                    pb = psb.tile([128, f], dt)
                    for c in range(0, f, 512):
                        nc.tensor.matmul(
                            pw[:, c : c + 512], sel_t[:], wbt[0:6, c : c + 512],
                            start=True, stop=True,
                        )
                        nc.tensor.matmul(
                            pb[:, c : c + 512], sel_t[:], wbt[0:6, f + c : f + c + 512],
                            start=True, stop=True,
                        )

                    ysub = yt[:, s2 * f : (s2 + 1) * f]
                    xsub = xt[:, s2 * f : (s2 + 1) * f]
                    nc.vector.tensor_mul(ysub, xsub, pw[:])
                    nc.vector.tensor_add(ysub, ysub, pb[:])

                nc.gpsimd.dma_start(out.ap()[bt], yt[:])

    nc.compile()
    return nc


_NC_CACHE = {}


def _get_nc():
    if "nc" not in _NC_CACHE:
        _NC_CACHE["nc"] = build_bass()
    return _NC_CACHE["nc"]


def _bf16_limbs(a: np.ndarray):
    """Exact 3-limb bf16 split: hi + mid + lo == a (fp32) bitwise."""
    import ml_dtypes

    a = a.astype(np.float32)
    hi = a.astype(ml_dtypes.bfloat16)
    r1 = a - hi.astype(np.float32)
    mid = r1.astype(ml_dtypes.bfloat16)
    r2 = r1 - mid.astype(np.float32)
    lo = r2.astype(ml_dtypes.bfloat16)
    return hi, mid, lo


def _pack_wb(w: np.ndarray, v: np.ndarray, s=S, f=F, w_run=W) -> np.ndarray:
    """Pack per-shard w/bias vectors as (ntiles, 6, 2f) bf16 limb rows:
    row 2l+k = limb l of partition-half k; cols [0:f] = w, [f:2f] = bias."""
    import ml_dtypes

    nbig = s // (2 * w_run)
    nsub = w_run // f
    ntiles = nbig * nsub
    wb = np.empty((nbig, nsub, 6, 2 * f), dtype=ml_dtypes.bfloat16)
    for vec, c0 in ((w, 0), (v, f)):
        limbs = _bf16_limbs(vec)
        for l in range(3):
            # big tile bt: half k of sub-tile s2 = vec[bt*2W + k*W + s2*f : +f]
            r = limbs[l].reshape(nbig, 2, nsub, f)  # (bt, k, s2, f)
            for k in range(2):
                wb[:, :, 2 * l + k, c0 : c0 + f] = r[:, k]
    return wb.reshape(ntiles, 6, 2 * f)


def kernel(x: np.ndarray, kernel: np.ndarray, bias: np.ndarray) -> np.ndarray:
    x = np.asarray(x, dtype=np.float32)
    ktab = np.asarray(kernel, dtype=np.float32)
    btab = np.asarray(bias, dtype=np.float32)
    assert x.shape == (B, N_FULL), x.shape

    w_full = _prime_mask(ktab, N_FULL)
    v_full = _prime_mask(btab, N_FULL)

    import ml_dtypes

    sel = np.zeros((6, 128), dtype=ml_dtypes.bfloat16)
    sel[0::2, 0:64] = 1.0
    sel[1::2, 64:128] = 1.0

    # Pre-permute x into per-core tile-major layout:
    # (core, bt, k, b, W) so each (128, W) DMA tile is contiguous DRAM.
    xt = np.ascontiguousarray(
        x.reshape(B, N_CORES, NBIG, 2, W).transpose(1, 2, 3, 0, 4)
    )

    in_maps = []
    for c in range(N_CORES):
        lo, hi = c * S, (c + 1) * S
        in_maps.append(
            {
                "x": xt[c].reshape(NBIG, 128, W),
                "wb": _pack_wb(w_full[lo:hi], v_full[lo:hi]),
                "sel": sel,
            }
        )

    nc = _get_nc()
    res = run_bass_kernel_spmd(
        nc,
        in_maps,
        core_ids=list(range(N_CORES)),
        trace=bool(os.environ.get("KERNEL_TRACE")),
    )
    # Inverse permute: (core, bt, k, b, W) -> (b, core*S + ...)
    ot = np.stack([r["out"].reshape(NBIG, 2, B, W) for r in res.results])
    out = np.ascontiguousarray(ot.transpose(3, 0, 1, 2, 4)).reshape(B, N_FULL)
    if os.environ.get("KERNEL_TRACE"):
        _NC_CACHE["last_exec_time_ns"] = res.exec_time_ns
        _NC_CACHE["last_results"] = res
    return out

